# revision 1
# baseline (speedup 1.0000x reference)
"""Multi-Head Latent Attention (GQA, causal) on 8 Trainium2 NeuronCores.

Sharding: tensor-parallel by heads. Core c owns query heads 4c..4c+3 and
kv head c. Each core computes:
  - its slice of the q projection (output dims c*512..(c+1)*512),
  - the full (small) kc/vc down-projections (replicated),
  - its kv head's up-projections (the reference's scrambled latent reshape
    is folded into strided access patterns on the compressed latents),
  - head-parallel causal attention with block skipping,
  - a partial o-projection (input dims c*512..(c+1)*512) over the full
    hidden size.
The host sums the 8 partial outputs (the all-reduce after o_proj).
"""

import sys

import ml_dtypes
import numpy as np

if "/opt/trn_rl_repo" not in sys.path:
    sys.path.insert(0, "/opt/trn_rl_repo")

B, S, HID = 1, 2048, 4096
H, HK, D = 32, 8, 128
L = D // 4  # 32
NCORE = 8
HPC = H // NCORE  # 4 query heads per core
NKT = HID // 128  # 32 k-tiles over hidden dim
CHUNK = 512
NCHUNK = S // CHUNK  # 4
NSQ = S // 128  # 16 sq tiles
NEG = -1e9

_BUILT = None


def _build():
    import concourse.mybir as mybir
    import concourse.tile as tile
    from concourse import bacc

    f32 = mybir.dt.float32
    bf16 = mybir.dt.bfloat16
    EXP = mybir.ActivationFunctionType.Exp

    nc = bacc.Bacc()

    ht = nc.dram_tensor("ht", [HID, S], bf16, kind="ExternalInput")
    htm = nc.dram_tensor("htm", [HID, S // NCORE], bf16, kind="ExternalInput")
    wqt = nc.dram_tensor("wqt", [HID, HPC * D], bf16, kind="ExternalInput")
    wkdt = nc.dram_tensor("wkdt", [HID, HK * L], bf16, kind="ExternalInput")
    wvdt = nc.dram_tensor("wvdt", [HID, HK * L], bf16, kind="ExternalInput")
    wkup = nc.dram_tensor("wkup", [128, 8 * D], bf16, kind="ExternalInput")
    wvup = nc.dram_tensor("wvup", [128, 8 * D], bf16, kind="ExternalInput")
    wot = nc.dram_tensor("wot", [HPC * D, HID], bf16, kind="ExternalInput")
    maskc = nc.dram_tensor("maskc", [128, 128], f32, kind="ExternalInput")
    ident = nc.dram_tensor("ident", [128, 128], bf16, kind="ExternalInput")
    outp = nc.dram_tensor("out", [S, HID], f32, kind="ExternalOutput")
    # kc/vc shard exchange: [p, tgt*512 + m*256 + u] per core -> gathered
    cv_bounce = nc.dram_tensor("cv_bounce", [128, 1024], bf16)
    cv_gath = nc.dram_tensor("cv_gath", [NCORE, 128, 1024], bf16, addr_space="Shared")

    with tile.TileContext(nc) as tc:
        with (
            tc.tile_pool(name="weights", bufs=1) as wpool,
            tc.tile_pool(name="persist", bufs=1) as ppool,
            tc.tile_pool(name="stream", bufs=4) as spool,
            tc.tile_pool(name="work", bufs=2) as kpool,
            tc.tile_pool(name="outs", bufs=3) as opool,
        ):
            # ---- resident weights (chunked loads so phase B starts early) ----
            wq_sb = wpool.tile([128, NKT, HPC * D], bf16)
            wkd_sb = wpool.tile([128, NKT, HK * L], bf16)
            wvd_sb = wpool.tile([128, NKT, HK * L], bf16)
            wq_r = wqt.rearrange("(k p) c -> p k c", p=128)
            wkd_r = wkdt.rearrange("(k p) c -> p k c", p=128)
            wvd_r = wvdt.rearrange("(k p) c -> p k c", p=128)
            # B0-critical weights first on the sync queue
            for g in range(8):
                ks = slice(g * 4, (g + 1) * 4)
                nc.sync.dma_start(out=wkd_sb[:, ks, :], in_=wkd_r[:, ks, :])
                nc.sync.dma_start(out=wvd_sb[:, ks, :], in_=wvd_r[:, ks, :])
            # q weights in parallel on the scalar queue
            for g in range(8):
                ks = slice(g * 4, (g + 1) * 4)
                nc.scalar.dma_start(out=wq_sb[:, ks, :], in_=wq_r[:, ks, :])
            wkup_sb = wpool.tile([128, 8 * D], bf16)
            nc.sync.dma_start(out=wkup_sb[:], in_=wkup[:])
            wvup_sb = wpool.tile([128, 8 * D], bf16)
            nc.sync.dma_start(out=wvup_sb[:], in_=wvup[:])
            mask_sb = wpool.tile([128, 128], f32)
            nc.sync.dma_start(out=mask_sb[:], in_=maskc[:])
            id_sb = wpool.tile([128, 128], bf16)
            nc.sync.dma_start(out=id_sb[:], in_=ident[:])
            wo_sb = wpool.tile([128, HPC, HID], bf16)
            wo_r = wot.rearrange("(k p) c -> p k c", p=128)
            for g in range(4):
                nc.scalar.dma_start(out=wo_sb[:, g, :], in_=wo_r[:, g, :])

            # ---- persistent activations ----
            qT = ppool.tile([128, HPC, S], bf16)  # [d, head, s]
            kcT = ppool.tile([128, 2, S], bf16)  # [latent%128, latent//128, s]
            vcT = ppool.tile([128, 2, S], bf16)
            kT = ppool.tile([128, S], bf16)  # [d, t] for our kv head
            v_sb = ppool.tile([128, NSQ, 128], bf16)  # [t%128, t//128, d]

            # ---- phase B: q projection, with the kc/vc shard + AllGather
            #      (B0) emitted after the first seq chunk so PE starts on
            #      DMA-ready q work while B0's inputs stream in ----
            SSH = S // NCORE  # 256
            with tc.tile_pool(name="psb", bufs=1, space="PSUM") as psb:

                def q_chunk(sc):
                    ps_q = [
                        psb.tile([128, CHUNK], f32, tag=f"ps_q{m}", name=f"ps_q{m}")
                        for m in range(HPC)
                    ]
                    for kp in range(NKT // 2):
                        hch = spool.tile([128, 2, CHUNK], bf16, tag="hch", name="hch")
                        nc.gpsimd.dma_start(
                            out=hch[:],
                            in_=ht.rearrange("(k p) s -> p k s", p=128)[
                                :, 2 * kp : 2 * kp + 2, sc * CHUNK : (sc + 1) * CHUNK
                            ],
                        )
                        for kk in range(2):
                            k = 2 * kp + kk
                            st = dict(start=(k == 0), stop=(k == NKT - 1))
                            rhs = hch[:, kk, :]
                            for m in range(HPC):
                                nc.tensor.matmul(
                                    ps_q[m][:],
                                    lhsT=wq_sb[:, k, m * 128 : (m + 1) * 128],
                                    rhs=rhs,
                                    **st,
                                )
                    cs = slice(sc * CHUNK, (sc + 1) * CHUNK)
                    for m in range(HPC):
                        (nc.scalar.copy if m % 2 else nc.vector.tensor_copy)(
                            qT[:, m, cs], ps_q[m][:]
                        )

                with (
                    tc.tile_pool(name="hmp", bufs=1) as hmp,
                    tc.tile_pool(name="psb0", bufs=2, space="PSUM") as psb0,
                ):
                    hm = hmp.tile([128, NKT, SSH], bf16)
                    hm_r = htm.rearrange("(k p) s -> p k s", p=128)
                    for g in range(4):
                        ks = slice(g * 8, (g + 1) * 8)
                        nc.gpsimd.dma_start(out=hm[:, ks, :], in_=hm_r[:, ks, :])
                    cvst = hmp.tile([128, 1024], bf16)
                    for ti, wsb_d in ((0, wkd_sb), (1, wvd_sb)):
                        for m in range(2):
                            ps_cv = psb0.tile(
                                [128, SSH], f32, tag="ps_cv", name="ps_cv"
                            )
                            for k in range(NKT):
                                nc.tensor.matmul(
                                    ps_cv[:],
                                    lhsT=wsb_d[:, k, m * 128 : (m + 1) * 128],
                                    rhs=hm[:, k, :],
                                    start=(k == 0),
                                    stop=(k == NKT - 1),
                                )
                            nc.any.tensor_copy(
                                cvst[:, ti * 512 + m * 256 : ti * 512 + (m + 1) * 256],
                                ps_cv[:],
                            )
                    nc.sync.dma_start(out=cv_bounce[:], in_=cvst[:])
                    nc.gpsimd.collective_compute(
                        "AllGather",
                        mybir.AluOpType.bypass,
                        replica_groups=[list(range(NCORE))],
                        ins=[cv_bounce[:]],
                        outs=[cv_gath[:]],
                    )
                    g_r = cv_gath.rearrange("r p (t m u) -> t p m r u", t=2, m=2)
                    for m in range(2):
                        nc.sync.dma_start(
                            out=kcT[:, m, :].rearrange("p (r u) -> p r u", r=NCORE),
                            in_=g_r[0, :, m],
                        )
                        nc.sync.dma_start(
                            out=vcT[:, m, :].rearrange("p (r u) -> p r u", r=NCORE),
                            in_=g_r[1, :, m],
                        )
                for sc in range(NCHUNK):
                    q_chunk(sc)

            # ---- phase C: up projections + v transpose ----
            # k_cmp[t, c'] with t = h*256+u, c' = r*64 + half*32 + j maps to
            #   (half==0 ? KC : VC)[8u + r (+4 for v_cmp), h*32 + j]
            # so rhs is a stride-8 slice of kcT/vcT along seq.
            kc_r = kcT.rearrange("p m (u r) -> p m r u", r=8)
            vc_r = vcT.rearrange("p m (u r) -> p m r u", r=8)
            with tc.tile_pool(name="psc", bufs=2, space="PSUM") as psc:
                for tgt, wsb, roff in ((kT, wkup_sb, 0), (None, wvup_sb, 4)):
                    vT = None
                    if tgt is None:
                        vT = ppool.tile([128, S], bf16, name="vT")
                        tgt = vT
                    for h in range(8):
                        ps_up = psc.tile([128, 256], f32, tag="ps_up", name="ps_up")
                        base = (h % 4) * 32
                        for blk in range(8):
                            r, half = blk // 2, blk % 2
                            src = kc_r if half == 0 else vc_r
                            nc.tensor.matmul(
                                ps_up[:],
                                lhsT=wsb[base : base + 32, blk * 128 : (blk + 1) * 128],
                                rhs=src[base : base + 32, h // 4, roff + r, :],
                                start=(blk == 0),
                                stop=(blk == 7),
                                tile_position=(base, 0),
                            )
                        nc.any.tensor_copy(tgt[:, h * 256 : (h + 1) * 256], ps_up[:])
                    if vT is not None:
                        for j in range(NSQ):
                            ps_tr = psc.tile([128, 128], bf16, tag="ps_tr", name="ps_tr")
                            nc.tensor.transpose(
                                ps_tr[:], vT[:, j * 128 : (j + 1) * 128], id_sb[:]
                            )
                            nc.any.tensor_copy(v_sb[:, j, :], ps_tr[:])

            # ---- phase D: attention + partial o-projection ----
            with (
                tc.tile_pool(name="ps_s", bufs=3, space="PSUM") as pss,
                tc.tile_pool(name="ps_sm", bufs=2, space="PSUM") as pssm,
                tc.tile_pool(name="ps_oo", bufs=2, space="PSUM") as psoo,
            ):
                at_prev = None
                i_prev = None

                def o_proj(i, at_t):
                    for n in range(HID // CHUNK):
                        ps_out = psoo.tile(
                            [128, CHUNK], f32, tag="ps_out", name="ps_out"
                        )
                        for hh in range(HPC):
                            nc.tensor.matmul(
                                ps_out[:],
                                lhsT=at_t[:, hh, :],
                                rhs=wo_sb[:, hh, n * CHUNK : (n + 1) * CHUNK],
                                start=(hh == 0),
                                stop=(hh == HPC - 1),
                            )
                        out_sb = opool.tile(
                            [128, CHUNK], f32, tag="out_sb", name="out_sb"
                        )
                        (nc.scalar.copy if n % 2 else nc.vector.tensor_copy)(
                            out_sb[:], ps_out[:]
                        )
                        nc.sync.dma_start(
                            out=outp[
                                i * 128 : (i + 1) * 128, n * CHUNK : (n + 1) * CHUNK
                            ],
                            in_=out_sb[:],
                        )

                for i in range(NSQ):
                    tlen = (i + 1) * 128
                    nch = (tlen + CHUNK - 1) // CHUNK
                    at_sb = kpool.tile([128, HPC, 128], bf16, tag="at_sb", name="at_sb")
                    p_l = []
                    s4_l = []
                    for h in range(HPC):
                        lhs_q = qT[:, h, i * 128 : (i + 1) * 128]
                        p_sb = kpool.tile(
                            [128, S], bf16, tag="p_sb", name="p_sb", bufs=4
                        )
                        sums4 = kpool.tile(
                            [128, 4], f32, tag="sums4", name="sums4", bufs=4
                        )
                        p_l.append(p_sb)
                        s4_l.append(sums4)
                        for c in range(nch):
                            lo = c * CHUNK
                            n = min(CHUNK, tlen - lo)
                            ps_s = pss.tile([128, CHUNK], f32, tag="ps_s", name="ps_s")
                            nc.tensor.matmul(
                                ps_s[:, :n],
                                lhsT=lhs_q,
                                rhs=kT[:, lo : lo + n],
                                start=True,
                                stop=True,
                            )
                            if c == nch - 1:
                                # causal mask on the diagonal block (chunk tail)
                                d0 = i * 128 - lo
                                nc.vector.tensor_add(
                                    ps_s[:, d0 : d0 + 128],
                                    ps_s[:, d0 : d0 + 128],
                                    mask_sb[:],
                                )
                            nc.scalar.activation(
                                p_sb[:, lo : lo + n],
                                ps_s[:, :n],
                                EXP,
                                accum_out=sums4[:, c : c + 1],
                            )
                    if at_prev is not None:
                        o_proj(i_prev, at_prev)
                    for h in range(HPC):
                        p_sb = p_l[h]
                        sums4 = s4_l[h]
                        sums = kpool.tile(
                            [128, 1], f32, tag="sums", name="sums", bufs=4
                        )
                        nc.vector.tensor_reduce(
                            sums[:],
                            sums4[:, :nch],
                            axis=mybir.AxisListType.X,
                            op=mybir.AluOpType.add,
                        )
                        recip = kpool.tile(
                            [128, 1], f32, tag="recip", name="recip", bufs=4
                        )
                        nc.vector.reciprocal(recip[:], sums[:])
                        # diag(1/sum): fold normalization into the p transpose
                        diag = kpool.tile(
                            [128, 128], bf16, tag="diag", name="diag", bufs=4
                        )
                        nc.vector.tensor_scalar_mul(diag[:], id_sb[:], recip[:])
                        pt_sb = kpool.tile([128, S], bf16, tag="pt_sb", name="pt_sb")
                        for jb in range(0, i + 1, 4):
                            nj = min(4, i + 1 - jb)
                            ps_pt = pssm.tile(
                                [128, CHUNK], f32, tag="ps_pt", name="ps_pt"
                            )
                            for jj in range(nj):
                                j = jb + jj
                                nc.tensor.matmul(
                                    ps_pt[:, jj * 128 : (jj + 1) * 128],
                                    lhsT=p_sb[:, j * 128 : (j + 1) * 128],
                                    rhs=diag[:],
                                    start=True,
                                    stop=True,
                                )
                            nc.vector.tensor_copy(
                                pt_sb[:, jb * 128 : (jb + nj) * 128],
                                ps_pt[:, : nj * 128],
                            )
                        ps_av = pssm.tile(
                            [128, 128], f32, tag="ps_av", name="ps_av", bufs=1
                        )
                        for j in range(i + 1):
                            nc.tensor.matmul(
                                ps_av[:],
                                lhsT=v_sb[:, j, :],
                                rhs=pt_sb[:, j * 128 : (j + 1) * 128],
                                start=(j == 0),
                                stop=(j == i),
                            )
                        nc.vector.tensor_copy(at_sb[:, h, :], ps_av[:])
                    at_prev = at_sb
                    i_prev = i
                o_proj(NSQ - 1, at_prev)
    nc.compile()
    return nc


def _prep_inputs(hidden_states, Wq, Wk_down, Wv_down, Wk_up, Wv_up, Wo):
    bf = ml_dtypes.bfloat16
    hs = np.asarray(hidden_states, dtype=np.float32).reshape(S, HID)
    ht = np.ascontiguousarray(hs.T).astype(bf)
    scale = np.float32(1.0) / np.sqrt(np.float32(D))
    Wq = np.asarray(Wq, dtype=np.float32)
    Wo = np.asarray(Wo, dtype=np.float32)
    wkdt = np.ascontiguousarray(np.asarray(Wk_down, np.float32).T).astype(bf)
    wvdt = np.ascontiguousarray(np.asarray(Wv_down, np.float32).T).astype(bf)
    mask = np.where(
        np.arange(128)[None, :] <= np.arange(128)[:, None], 0.0, NEG
    ).astype(np.float32)
    identity = np.eye(128, dtype=bf)

    def up_blocks(w):  # w: (128, 256) rows of Wk_up/Wv_up for this core
        arr = np.zeros((128, 8 * 128), np.float32)
        for r in range(4):
            for half in range(2):
                blk = r * 2 + half
                bT = w[:, r * 64 + half * 32 : r * 64 + half * 32 + 32].T
                for b in range(4):
                    arr[b * 32 : (b + 1) * 32, blk * 128 : (blk + 1) * 128] = bT
        return arr.astype(bf)

    in_maps = []
    for c in range(NCORE):
        htm = np.ascontiguousarray(ht[:, c * (S // NCORE) : (c + 1) * (S // NCORE)])
        wqt = np.ascontiguousarray((Wq[c * 512 : (c + 1) * 512, :] * scale).T).astype(bf)
        wkup = up_blocks(np.asarray(Wk_up[c * 128 : (c + 1) * 128, :], np.float32))
        wvup = up_blocks(np.asarray(Wv_up[c * 128 : (c + 1) * 128, :], np.float32))
        wot = np.ascontiguousarray(Wo[:, c * 512 : (c + 1) * 512].T).astype(bf)
        in_maps.append(
            dict(
                ht=ht,
                htm=htm,
                wqt=wqt,
                wkdt=wkdt,
                wvdt=wvdt,
                wkup=wkup,
                wvup=wvup,
                wot=wot,
                maskc=mask,
                ident=identity,
            )
        )
    return in_maps


def run(trace=False, **inputs):
    from concourse.bass_utils import run_bass_kernel_spmd

    global _BUILT
    if _BUILT is None:
        _BUILT = _build()
    in_maps = _prep_inputs(**inputs)
    res = run_bass_kernel_spmd(
        _BUILT, in_maps, core_ids=list(range(NCORE)), trace=trace
    )
    acc = np.array(res.results[0]["out"], dtype=np.float32, copy=True)
    for r in res.results[1:]:
        acc += np.asarray(r["out"], dtype=np.float32)
    return acc.reshape(B, S, HID), res


def kernel(**inputs):
    out, _ = run(trace=False, **inputs)
    return out



# revision 5
# speedup vs baseline: 1.0201x; 1.0201x over previous
"""Multi-Head Latent Attention (GQA, causal) on 8 Trainium2 NeuronCores.

Sharding: tensor-parallel by heads. Core c owns query heads 4c..4c+3 and
kv head c. Each core computes:
  - its slice of the q projection (output dims c*512..(c+1)*512),
  - its S/8 sequence shard of the kc/vc down-projections, AllGathered so
    every core sees the full compressed latents (the reference's scrambled
    latent reshape is folded into strided access patterns),
  - its kv head's up-projections (k in [d,t] layout, v directly in [t,d]),
  - head-parallel causal attention computed TRANSPOSED: scores^T[t,q] come
    straight out of the PE in the layout the AV matmul consumes, so no
    per-block P transposes are needed. Softmax denominators are N=1
    ones-matmuls; normalization is folded into the AV output copy, and one
    128x128 PE transpose per (head, q-tile) restores [d,q] for o_proj,
  - a partial o-projection (input dims c*512..(c+1)*512) over the full
    hidden size.
The host sums the 8 partial outputs (the all-reduce after o_proj).
"""

import sys

import ml_dtypes
import numpy as np

if "/opt/trn_rl_repo" not in sys.path:
    sys.path.insert(0, "/opt/trn_rl_repo")

B, S, HID = 1, 2048, 4096
H, HK, D = 32, 8, 128
L = D // 4  # 32
NCORE = 8
HPC = H // NCORE  # 4 query heads per core
NKT = HID // 128  # 32 k-tiles over hidden dim
CHUNK = 512
NCHUNK = S // CHUNK  # 4
NSQ = S // 128  # 16 sq tiles
SSH = S // NCORE  # 256
NEG = -1e9

_BUILT = None


def _build():
    import concourse.mybir as mybir
    import concourse.tile as tile
    from concourse import bacc

    f32 = mybir.dt.float32
    bf16 = mybir.dt.bfloat16
    EXP = mybir.ActivationFunctionType.Exp

    nc = bacc.Bacc()

    ht = nc.dram_tensor("ht", [HID, S], bf16, kind="ExternalInput")
    htm = nc.dram_tensor("htm", [HID, SSH], bf16, kind="ExternalInput")
    wqt = nc.dram_tensor("wqt", [HID, HPC * D], bf16, kind="ExternalInput")
    wkdt = nc.dram_tensor("wkdt", [HID, HK * L], bf16, kind="ExternalInput")
    wvdt = nc.dram_tensor("wvdt", [HID, HK * L], bf16, kind="ExternalInput")
    wkup = nc.dram_tensor("wkup", [128, 8 * D], bf16, kind="ExternalInput")
    wvup = nc.dram_tensor("wvup", [128, 8 * D], bf16, kind="ExternalInput")
    wot = nc.dram_tensor("wot", [HPC * D, HID], bf16, kind="ExternalInput")
    maskt = nc.dram_tensor("maskt", [128, 128], f32, kind="ExternalInput")
    ident = nc.dram_tensor("ident", [128, 128], bf16, kind="ExternalInput")
    outp = nc.dram_tensor("out", [S, HID], f32, kind="ExternalOutput")
    # kc/vc shard exchange: [p, tgt*512 + m*256 + u] per core -> gathered
    cv_bounce = nc.dram_tensor("cv_bounce", [128, 1024], bf16)
    cv_gath = nc.dram_tensor("cv_gath", [NCORE, 128, 1024], bf16, addr_space="Shared")

    with tile.TileContext(nc) as tc:
        with (
            tc.tile_pool(name="weights", bufs=1) as wpool,
            tc.tile_pool(name="persist", bufs=1) as ppool,
            tc.tile_pool(name="stream", bufs=6) as spool,
            tc.tile_pool(name="outs", bufs=3) as opool,
        ):
            # ---- constants + resident weights ----
            ones_sb = wpool.tile([128, 1], bf16)
            nc.gpsimd.memset(ones_sb[:], 1.0)
            mask_sb = wpool.tile([128, 128], f32)
            nc.scalar.dma_start(out=mask_sb[:], in_=maskt[:])
            id_sb = wpool.tile([128, 128], bf16)
            nc.scalar.dma_start(out=id_sb[:], in_=ident[:])
            wkup_sb = wpool.tile([128, 8 * D], bf16)
            nc.scalar.dma_start(out=wkup_sb[:], in_=wkup[:])
            wvup_sb = wpool.tile([128, 8 * D], bf16)
            nc.scalar.dma_start(out=wvup_sb[:], in_=wvup[:])
            wq_sb = wpool.tile([128, NKT, HPC * D], bf16)
            wq_r = wqt.rearrange("(k p) c -> p k c", p=128)
            for g in range(8):
                ks = slice(g * 4, (g + 1) * 4)
                nc.scalar.dma_start(out=wq_sb[:, ks, :], in_=wq_r[:, ks, :])
            wo_sb = wpool.tile([128, HPC, HID], bf16)
            wo_r = wot.rearrange("(k p) c -> p k c", p=128)

            # ---- persistent activations ----
            qT = ppool.tile([128, HPC, S], bf16)  # [d, head, s]
            kcT = ppool.tile([128, 2, S], bf16)  # [latent%128, latent//128, s]
            vcT = ppool.tile([128, 2, S], bf16)
            kT = ppool.tile([128, S], bf16)  # [d, t] for our kv head
            v_sb = ppool.tile([128, NSQ, 128], bf16)  # [t%128, t//128, d]

            ht_r = ht.rearrange("(k p) s -> p k s", p=128)

            # ---- phase B0: this core's kc/vc seq-shard + AllGather ----
            with (
                tc.tile_pool(name="b0", bufs=1) as bpool,
                tc.tile_pool(name="psb0", bufs=1, space="PSUM") as psb0,
            ):
                wkd_sb = bpool.tile([128, NKT, HK * L], bf16)
                wvd_sb = bpool.tile([128, NKT, HK * L], bf16)
                wkd_r = wkdt.rearrange("(k p) c -> p k c", p=128)
                wvd_r = wvdt.rearrange("(k p) c -> p k c", p=128)
                for g in range(8):
                    ks = slice(g * 4, (g + 1) * 4)
                    nc.sync.dma_start(out=wkd_sb[:, ks, :], in_=wkd_r[:, ks, :])
                    nc.sync.dma_start(out=wvd_sb[:, ks, :], in_=wvd_r[:, ks, :])
                hm = bpool.tile([128, NKT, SSH], bf16)
                hm_r = htm.rearrange("(k p) s -> p k s", p=128)
                for g in range(4):
                    ks = slice(g * 8, (g + 1) * 8)
                    nc.gpsimd.dma_start(out=hm[:, ks, :], in_=hm_r[:, ks, :])
                # 4 accumulators (kc/vc x latent-half), k outer so the MMs
                # stream behind the chunked weight loads
                ps_cv = [
                    psb0.tile([128, SSH], f32, tag=f"ps_cv{t}", name=f"ps_cv{t}")
                    for t in range(4)
                ]
                for k in range(NKT):
                    for ti, wsb_d in ((0, wkd_sb), (1, wvd_sb)):
                        for m in range(2):
                            nc.tensor.matmul(
                                ps_cv[ti * 2 + m][:],
                                lhsT=wsb_d[:, k, m * 128 : (m + 1) * 128],
                                rhs=hm[:, k, :],
                                start=(k == 0),
                                stop=(k == NKT - 1),
                            )
                cvst = bpool.tile([128, 1024], bf16)
                for t in range(4):
                    ti, m = t // 2, t % 2
                    eng = nc.vector.tensor_copy if t % 2 == 0 else nc.scalar.copy
                    eng(
                        cvst[:, ti * 512 + m * 256 : ti * 512 + (m + 1) * 256],
                        ps_cv[t][:],
                    )
                nc.sync.dma_start(out=cv_bounce[:], in_=cvst[:])

            # ---- phase B: q projection, with the collective emitted after
            #      chunk 0 so it doesn't block the gpsimd ht-streaming queue,
            #      and PE chews q-proj while the AllGather flies ----
            with tc.tile_pool(name="psq", bufs=1, space="PSUM") as psq:

                def q_chunk(sc):
                    ps_q = [
                        psq.tile([128, CHUNK], f32, tag=f"ps_q{m}", name=f"ps_q{m}")
                        for m in range(HPC)
                    ]
                    for kp in range(NKT // 2):
                        hch = spool.tile([128, 2, CHUNK], bf16, tag="hch", name="hch")
                        nc.gpsimd.dma_start(
                            out=hch[:],
                            in_=ht_r[
                                :, 2 * kp : 2 * kp + 2, sc * CHUNK : (sc + 1) * CHUNK
                            ],
                        )
                        for kk in range(2):
                            k = 2 * kp + kk
                            st = dict(start=(k == 0), stop=(k == NKT - 1))
                            for m in range(HPC):
                                nc.tensor.matmul(
                                    ps_q[m][:],
                                    lhsT=wq_sb[:, k, m * 128 : (m + 1) * 128],
                                    rhs=hch[:, kk, :],
                                    **st,
                                )
                    cs = slice(sc * CHUNK, (sc + 1) * CHUNK)
                    for m in range(HPC):
                        (nc.scalar.copy if m % 2 else nc.vector.tensor_copy)(
                            qT[:, m, cs], ps_q[m][:]
                        )

                q_chunk(0)
                nc.gpsimd.collective_compute(
                    "AllGather",
                    mybir.AluOpType.bypass,
                    replica_groups=[list(range(NCORE))],
                    ins=[cv_bounce[:]],
                    outs=[cv_gath[:]],
                )
                g_r = cv_gath.rearrange("r p (t m u) -> t p m r u", t=2, m=2)
                for m in range(2):
                    nc.sync.dma_start(
                        out=kcT[:, m, :].rearrange("p (r u) -> p r u", r=NCORE),
                        in_=g_r[0, :, m],
                    )
                    nc.sync.dma_start(
                        out=vcT[:, m, :].rearrange("p (r u) -> p r u", r=NCORE),
                        in_=g_r[1, :, m],
                    )
                # wo behind the gathers on the sync queue: issues once the
                # collective lands, transfers while the DMA engines are quiet
                for g in range(HPC):
                    nc.sync.dma_start(out=wo_sb[:, g, :], in_=wo_r[:, g, :])
                for sc in range(1, NCHUNK):
                    q_chunk(sc)

            # ---- phase C: up projections ----
            # k_cmp[t, c'] with t = h*256+u, c' = r*64 + half*32 + j maps to
            #   (half==0 ? KC : VC)[8u + r (+4 for v_cmp), h*32 + j]
            # so the latent operand is a stride-8 slice of kcT/vcT along seq.
            kc_r = kcT.rearrange("p m (u r) -> p m r u", r=8)
            vc_r = vcT.rearrange("p m (u r) -> p m r u", r=8)
            with tc.tile_pool(name="psc", bufs=2, space="PSUM") as psc:
                # k: [d, t] (weights stationary)
                for h in range(8):
                    base = (h % 4) * 32
                    ps_up = psc.tile([128, 256], f32, tag="ps_up", name="ps_up")
                    for blk in range(8):
                        r, half = blk // 2, blk % 2
                        src = kc_r if half == 0 else vc_r
                        nc.tensor.matmul(
                            ps_up[:],
                            lhsT=wkup_sb[base : base + 32, blk * 128 : (blk + 1) * 128],
                            rhs=src[base : base + 32, h // 4, r, :],
                            start=(blk == 0),
                            stop=(blk == 7),
                            tile_position=(base, 0),
                        )
                    (nc.vector.tensor_copy if h % 2 else nc.scalar.copy)(
                        kT[:, h * 256 : (h + 1) * 256], ps_up[:]
                    )
                # v: directly [t, d] (latents stationary) - no transposes
                for tt in range(NSQ):
                    h, ub = tt // 2, tt % 2
                    base = (h % 4) * 32
                    ps_vt = psc.tile([128, 128], f32, tag="ps_vt", name="ps_vt")
                    for blk in range(8):
                        r, half = blk // 2, blk % 2
                        src = kc_r if half == 0 else vc_r
                        nc.tensor.matmul(
                            ps_vt[:],
                            lhsT=src[
                                base : base + 32, h // 4, 4 + r,
                                ub * 128 : (ub + 1) * 128,
                            ],
                            rhs=wvup_sb[base : base + 32, blk * 128 : (blk + 1) * 128],
                            start=(blk == 0),
                            stop=(blk == 7),
                            tile_position=(base, 0),
                        )
                    (nc.vector.tensor_copy if tt % 2 else nc.scalar.copy)(
                        v_sb[:, tt, :], ps_vt[:]
                    )

            # ---- phase D: transposed attention + partial o-projection ----
            with (
                tc.tile_pool(name="pt", bufs=1) as ptpool,
                tc.tile_pool(name="attn", bufs=2) as apool,
                tc.tile_pool(name="pss", bufs=2, space="PSUM") as pss,
                tc.tile_pool(name="pssum", bufs=1, space="PSUM") as pssum,
                tc.tile_pool(name="psav", bufs=2, space="PSUM") as psav,
                tc.tile_pool(name="pso", bufs=2, space="PSUM") as pso,
            ):
                for c in range(NCHUNK):
                    nj = 4 * c + 4
                    at_sb = apool.tile(
                        [128, HPC, CHUNK], bf16, tag="at_sb", name="at_sb"
                    )
                    for h in range(HPC):
                        # scores^T[t, q] per 128-t-block over this q-chunk
                        pts = []
                        for j in range(nj):
                            q0 = max(0, 128 * (j - 4 * c))
                            ps_s = pss.tile([128, CHUNK], f32, tag="ps_s", name="ps_s")
                            nc.tensor.matmul(
                                ps_s[:, q0:CHUNK],
                                lhsT=kT[:, j * 128 : (j + 1) * 128],
                                rhs=qT[:, h, c * CHUNK + q0 : (c + 1) * CHUNK],
                                start=True,
                                stop=True,
                            )
                            if j >= 4 * c:
                                # causal mask on the diagonal 128-block
                                nc.vector.tensor_add(
                                    ps_s[:, q0 : q0 + 128],
                                    ps_s[:, q0 : q0 + 128],
                                    mask_sb[:],
                                )
                            ptile = ptpool.tile(
                                [128, CHUNK], bf16, tag=f"pt{j}", name=f"pt{j}"
                            )
                            nc.scalar.activation(
                                ptile[:, q0:CHUNK], ps_s[:, q0:CHUNK], EXP
                            )
                            pts.append(ptile)
                        for qt in range(4):
                            i = 4 * c + qt
                            qs = slice(qt * 128, (qt + 1) * 128)
                            # softmax denominators: cheap N=1 ones-matmuls
                            ps_sum = pssum.tile(
                                [128, 1], f32, tag="ps_sum", name="ps_sum"
                            )
                            for j in range(i + 1):
                                nc.tensor.matmul(
                                    ps_sum[:],
                                    lhsT=pts[j][:, qs],
                                    rhs=ones_sb[:],
                                    start=(j == 0),
                                    stop=(j == i),
                                )
                            rec = apool.tile([128, 1], f32, tag="rec", name="rec")
                            nc.vector.reciprocal(rec[:], ps_sum[:])
                            # AV in [q, d] so the normalization is a
                            # per-partition scale on the PSUM->SBUF copy
                            ps_av = psav.tile(
                                [128, 128], f32, tag="ps_av", name="ps_av"
                            )
                            for j in range(i + 1):
                                nc.tensor.matmul(
                                    ps_av[:],
                                    lhsT=pts[j][:, qs],
                                    rhs=v_sb[:, j, :],
                                    start=(j == 0),
                                    stop=(j == i),
                                )
                            at_qd = apool.tile(
                                [128, 128], bf16, tag="at_qd", name="at_qd"
                            )
                            nc.vector.tensor_scalar_mul(at_qd[:], ps_av[:], rec[:])
                            ps_tr = psav.tile(
                                [128, 128], bf16, tag="ps_tr", name="ps_tr", bufs=1
                            )
                            nc.tensor.transpose(ps_tr[:], at_qd[:], id_sb[:])
                            (nc.scalar.copy if qt % 2 else nc.vector.tensor_copy)(
                                at_sb[:, h, qs], ps_tr[:]
                            )
                    # o-projection for this chunk's 4 q-tiles
                    for qt in range(4):
                        i = 4 * c + qt
                        for pn in range(HID // (2 * CHUNK)):
                            out_sb = opool.tile(
                                [128, 2 * CHUNK], f32, tag="out_sb", name="out_sb"
                            )
                            for half in range(2):
                                n = 2 * pn + half
                                ps_o = pso.tile(
                                    [128, CHUNK], f32, tag="ps_o", name="ps_o"
                                )
                                for hh in range(HPC):
                                    nc.tensor.matmul(
                                        ps_o[:],
                                        lhsT=at_sb[:, hh, qt * 128 : (qt + 1) * 128],
                                        rhs=wo_sb[:, hh, n * CHUNK : (n + 1) * CHUNK],
                                        start=(hh == 0),
                                        stop=(hh == HPC - 1),
                                    )
                                (nc.scalar.copy if half else nc.vector.tensor_copy)(
                                    out_sb[:, half * CHUNK : (half + 1) * CHUNK],
                                    ps_o[:],
                                )
                            (nc.sync if pn % 2 == 0 else nc.gpsimd).dma_start(
                                out=outp[
                                    i * 128 : (i + 1) * 128,
                                    2 * pn * CHUNK : 2 * (pn + 1) * CHUNK,
                                ],
                                in_=out_sb[:],
                            )
    nc.compile()
    return nc


def _prep_inputs(hidden_states, Wq, Wk_down, Wv_down, Wk_up, Wv_up, Wo):
    bf = ml_dtypes.bfloat16
    hs = np.asarray(hidden_states, dtype=np.float32).reshape(S, HID)
    ht = np.ascontiguousarray(hs.T).astype(bf)
    scale = np.float32(1.0) / np.sqrt(np.float32(D))
    Wq = np.asarray(Wq, dtype=np.float32)
    Wo = np.asarray(Wo, dtype=np.float32)
    wkdt = np.ascontiguousarray(np.asarray(Wk_down, np.float32).T).astype(bf)
    wvdt = np.ascontiguousarray(np.asarray(Wv_down, np.float32).T).astype(bf)
    # transposed causal mask: rows t, cols q; allowed where q >= t
    mask = np.where(
        np.arange(128)[None, :] >= np.arange(128)[:, None], 0.0, NEG
    ).astype(np.float32)
    identity = np.eye(128, dtype=bf)

    def up_blocks(w):  # w: (128, 256) rows of Wk_up/Wv_up for this core
        arr = np.zeros((128, 8 * 128), np.float32)
        for r in range(4):
            for half in range(2):
                blk = r * 2 + half
                bT = w[:, r * 64 + half * 32 : r * 64 + half * 32 + 32].T
                for b in range(4):
                    arr[b * 32 : (b + 1) * 32, blk * 128 : (blk + 1) * 128] = bT
        return arr.astype(bf)

    in_maps = []
    for c in range(NCORE):
        htm = np.ascontiguousarray(ht[:, c * SSH : (c + 1) * SSH])
        wqt = np.ascontiguousarray((Wq[c * 512 : (c + 1) * 512, :] * scale).T).astype(
            bf
        )
        wkup = up_blocks(np.asarray(Wk_up[c * 128 : (c + 1) * 128, :], np.float32))
        wvup = up_blocks(np.asarray(Wv_up[c * 128 : (c + 1) * 128, :], np.float32))
        wot = np.ascontiguousarray(Wo[:, c * 512 : (c + 1) * 512].T).astype(bf)
        in_maps.append(
            dict(
                ht=ht,
                htm=htm,
                wqt=wqt,
                wkdt=wkdt,
                wvdt=wvdt,
                wkup=wkup,
                wvup=wvup,
                wot=wot,
                maskt=mask,
                ident=identity,
            )
        )
    return in_maps


def run(trace=False, **inputs):
    from concourse.bass_utils import run_bass_kernel_spmd

    global _BUILT
    if _BUILT is None:
        _BUILT = _build()
    in_maps = _prep_inputs(**inputs)
    res = run_bass_kernel_spmd(
        _BUILT, in_maps, core_ids=list(range(NCORE)), trace=trace
    )
    acc = np.array(res.results[0]["out"], dtype=np.float32, copy=True)
    for r in res.results[1:]:
        acc += np.asarray(r["out"], dtype=np.float32)
    return acc.reshape(B, S, HID), res


def kernel(**inputs):
    out, _ = run(trace=False, **inputs)
    return out


# revision 7
# speedup vs baseline: 1.0658x; 1.0448x over previous
"""Multi-Head Latent Attention (GQA, causal) on 8 Trainium2 NeuronCores.

Sharding: tensor-parallel by heads. Core c owns query heads 4c..4c+3 and
kv head c. Each core computes:
  - its slice of the q projection (output dims c*512..(c+1)*512),
  - its S/8 sequence shard of the kc/vc down-projections, AllGathered so
    every core sees the full compressed latents (the reference's scrambled
    latent reshape is folded into strided access patterns),
  - its kv head's up-projections (k in [d,t] layout, v directly in [t,d]),
  - head-parallel causal attention computed TRANSPOSED: scores^T[t,q] come
    straight out of the PE in the layout the AV matmul consumes, so no
    per-block P transposes are needed. Softmax denominators are N=1
    ones-matmuls; normalization is folded into the AV output copy, and one
    128x128 PE transpose per (head, q-tile) restores [d,q] for o_proj,
  - a partial o-projection (input dims c*512..(c+1)*512) over the full
    hidden size.
The host sums the 8 partial outputs (the all-reduce after o_proj).
"""

import sys

import ml_dtypes
import numpy as np

if "/opt/trn_rl_repo" not in sys.path:
    sys.path.insert(0, "/opt/trn_rl_repo")

B, S, HID = 1, 2048, 4096
H, HK, D = 32, 8, 128
L = D // 4  # 32
NCORE = 8
HPC = H // NCORE  # 4 query heads per core
NKT = HID // 128  # 32 k-tiles over hidden dim
CHUNK = 512
NCHUNK = S // CHUNK  # 4
NSQ = S // 128  # 16 sq tiles
SSH = S // NCORE  # 256
NEG = -1e9

_BUILT = None


def _build():
    import concourse.mybir as mybir
    import concourse.tile as tile
    from concourse import bacc

    f32 = mybir.dt.float32
    bf16 = mybir.dt.bfloat16
    EXP = mybir.ActivationFunctionType.Exp

    nc = bacc.Bacc()

    ht = nc.dram_tensor("ht", [HID, S], bf16, kind="ExternalInput")
    htm = nc.dram_tensor("htm", [HID, SSH], bf16, kind="ExternalInput")
    wqt = nc.dram_tensor("wqt", [HID, HPC * D], bf16, kind="ExternalInput")
    wkdt = nc.dram_tensor("wkdt", [HID, HK * L], bf16, kind="ExternalInput")
    wvdt = nc.dram_tensor("wvdt", [HID, HK * L], bf16, kind="ExternalInput")
    wkup = nc.dram_tensor("wkup", [128, 8 * D], bf16, kind="ExternalInput")
    wvup = nc.dram_tensor("wvup", [128, 8 * D], bf16, kind="ExternalInput")
    wot = nc.dram_tensor("wot", [HPC * D, HID], bf16, kind="ExternalInput")
    maskt = nc.dram_tensor("maskt", [128, 128], f32, kind="ExternalInput")
    ident = nc.dram_tensor("ident", [128, 128], bf16, kind="ExternalInput")
    outp = nc.dram_tensor("out", [S, HID], f32, kind="ExternalOutput")
    # kc/vc shard exchange: [p, tgt*512 + m*256 + u] per core -> gathered
    cv_bounce = nc.dram_tensor("cv_bounce", [128, 1024], bf16)
    cv_gath = nc.dram_tensor("cv_gath", [NCORE, 128, 1024], bf16, addr_space="Shared")

    with tile.TileContext(nc) as tc:
        with (
            tc.tile_pool(name="weights", bufs=1) as wpool,
            tc.tile_pool(name="persist", bufs=1) as ppool,
            tc.tile_pool(name="stream", bufs=6) as spool,
            tc.tile_pool(name="outs", bufs=3) as opool,
        ):
            # ---- constants + resident weights ----
            ones_sb = wpool.tile([128, 1], bf16)
            nc.gpsimd.memset(ones_sb[:], 1.0)
            mask_sb = wpool.tile([128, 128], f32)
            nc.scalar.dma_start(out=mask_sb[:], in_=maskt[:])
            id_sb = wpool.tile([128, 128], bf16)
            nc.scalar.dma_start(out=id_sb[:], in_=ident[:])
            wkup_sb = wpool.tile([128, 8 * D], bf16)
            nc.scalar.dma_start(out=wkup_sb[:], in_=wkup[:])
            wvup_sb = wpool.tile([128, 8 * D], bf16)
            nc.scalar.dma_start(out=wvup_sb[:], in_=wvup[:])
            wq_sb = wpool.tile([128, NKT, HPC * D], bf16)
            wq_r = wqt.rearrange("(k p) c -> p k c", p=128)
            for g in range(2):
                ks = slice(g * 4, (g + 1) * 4)
                nc.scalar.dma_start(out=wq_sb[:, ks, :], in_=wq_r[:, ks, :])
            wo_sb = wpool.tile([128, HPC, HID], bf16)
            wo_r = wot.rearrange("(k p) c -> p k c", p=128)

            # ---- persistent activations ----
            qT = ppool.tile([128, HPC, S], bf16)  # [d, head, s]
            kcT = ppool.tile([128, 2, S], bf16)  # [latent%128, latent//128, s]
            vcT = ppool.tile([128, 2, S], bf16)
            kT = ppool.tile([128, S], bf16)  # [d, t] for our kv head
            v_sb = ppool.tile([128, NSQ, 128], bf16)  # [t%128, t//128, d]

            ht_r = ht.rearrange("(k p) s -> p k s", p=128)

            # ---- phase B0: this core's kc/vc seq-shard + AllGather ----
            with (
                tc.tile_pool(name="b0", bufs=1) as bpool,
                tc.tile_pool(name="psb0", bufs=1, space="PSUM") as psb0,
            ):
                wkd_sb = bpool.tile([128, NKT, HK * L], bf16)
                wvd_sb = bpool.tile([128, NKT, HK * L], bf16)
                wkd_r = wkdt.rearrange("(k p) c -> p k c", p=128)
                wvd_r = wvdt.rearrange("(k p) c -> p k c", p=128)
                for g in range(8):
                    ks = slice(g * 4, (g + 1) * 4)
                    nc.sync.dma_start(out=wkd_sb[:, ks, :], in_=wkd_r[:, ks, :])
                    nc.sync.dma_start(out=wvd_sb[:, ks, :], in_=wvd_r[:, ks, :])
                hm = bpool.tile([128, NKT, SSH], bf16)
                hm_r = htm.rearrange("(k p) s -> p k s", p=128)
                for g in range(4):
                    ks = slice(g * 8, (g + 1) * 8)
                    nc.gpsimd.dma_start(out=hm[:, ks, :], in_=hm_r[:, ks, :])
                # 4 accumulators (kc/vc x latent-half), k outer so the MMs
                # stream behind the chunked weight loads
                ps_cv = [
                    psb0.tile([128, SSH], f32, tag=f"ps_cv{t}", name=f"ps_cv{t}")
                    for t in range(4)
                ]
                for k in range(NKT):
                    for ti, wsb_d in ((0, wkd_sb), (1, wvd_sb)):
                        for m in range(2):
                            nc.tensor.matmul(
                                ps_cv[ti * 2 + m][:],
                                lhsT=wsb_d[:, k, m * 128 : (m + 1) * 128],
                                rhs=hm[:, k, :],
                                start=(k == 0),
                                stop=(k == NKT - 1),
                            )
                cvst = bpool.tile([128, 1024], bf16)
                for t in range(4):
                    ti, m = t // 2, t % 2
                    eng = nc.vector.tensor_copy if t % 2 == 0 else nc.scalar.copy
                    eng(
                        cvst[:, ti * 512 + m * 256 : ti * 512 + (m + 1) * 256],
                        ps_cv[t][:],
                    )
                nc.sync.dma_start(out=cv_bounce[:], in_=cvst[:])
            for g in range(2, 8):
                ks = slice(g * 4, (g + 1) * 4)
                nc.scalar.dma_start(out=wq_sb[:, ks, :], in_=wq_r[:, ks, :])

            # ---- phase B: q projection, with the collective emitted after
            #      chunk 0 so it doesn't block the gpsimd ht-streaming queue,
            #      and PE chews q-proj while the AllGather flies ----
            with tc.tile_pool(name="psq", bufs=1, space="PSUM") as psq:

                def q_chunk(sc):
                    ps_q = [
                        psq.tile([128, CHUNK], f32, tag=f"ps_q{m}", name=f"ps_q{m}")
                        for m in range(HPC)
                    ]
                    for kp in range(NKT // 2):
                        hch = spool.tile([128, 2, CHUNK], bf16, tag="hch", name="hch")
                        nc.gpsimd.dma_start(
                            out=hch[:],
                            in_=ht_r[
                                :, 2 * kp : 2 * kp + 2, sc * CHUNK : (sc + 1) * CHUNK
                            ],
                        )
                        for kk in range(2):
                            k = 2 * kp + kk
                            st = dict(start=(k == 0), stop=(k == NKT - 1))
                            for m in range(HPC):
                                nc.tensor.matmul(
                                    ps_q[m][:],
                                    lhsT=wq_sb[:, k, m * 128 : (m + 1) * 128],
                                    rhs=hch[:, kk, :],
                                    **st,
                                )
                    cs = slice(sc * CHUNK, (sc + 1) * CHUNK)
                    for m in range(HPC):
                        (nc.scalar.copy if m % 2 else nc.vector.tensor_copy)(
                            qT[:, m, cs], ps_q[m][:]
                        )

                q_chunk(0)
                nc.gpsimd.collective_compute(
                    "AllGather",
                    mybir.AluOpType.bypass,
                    replica_groups=[list(range(NCORE))],
                    ins=[cv_bounce[:]],
                    outs=[cv_gath[:]],
                )
                g_r = cv_gath.rearrange("r p (t m u) -> t p m r u", t=2, m=2)
                for m in range(2):
                    nc.sync.dma_start(
                        out=kcT[:, m, :].rearrange("p (r u) -> p r u", r=NCORE),
                        in_=g_r[0, :, m],
                    )
                    nc.sync.dma_start(
                        out=vcT[:, m, :].rearrange("p (r u) -> p r u", r=NCORE),
                        in_=g_r[1, :, m],
                    )
                # wo behind the gathers on the sync queue: issues once the
                # collective lands, transfers while the DMA engines are quiet
                for g in range(HPC):
                    nc.sync.dma_start(out=wo_sb[:, g, :], in_=wo_r[:, g, :])
                for sc in range(1, NCHUNK):
                    q_chunk(sc)

            # ---- phase C: up projections ----
            # k_cmp[t, c'] with t = h*256+u, c' = r*64 + half*32 + j maps to
            #   (half==0 ? KC : VC)[8u + r (+4 for v_cmp), h*32 + j]
            # so the latent operand is a stride-8 slice of kcT/vcT along seq.
            kc_r = kcT.rearrange("p m (u r) -> p m r u", r=8)
            vc_r = vcT.rearrange("p m (u r) -> p m r u", r=8)
            with tc.tile_pool(name="psc", bufs=2, space="PSUM") as psc:
                # k: [d, t] (weights stationary)
                for h in range(8):
                    base = (h % 4) * 32
                    ps_up = psc.tile([128, 256], f32, tag="ps_up", name="ps_up")
                    for blk in range(8):
                        r, half = blk // 2, blk % 2
                        src = kc_r if half == 0 else vc_r
                        nc.tensor.matmul(
                            ps_up[:],
                            lhsT=wkup_sb[base : base + 32, blk * 128 : (blk + 1) * 128],
                            rhs=src[base : base + 32, h // 4, r, :],
                            start=(blk == 0),
                            stop=(blk == 7),
                            tile_position=(base, 0),
                        )
                    (nc.vector.tensor_copy if h % 2 else nc.scalar.copy)(
                        kT[:, h * 256 : (h + 1) * 256], ps_up[:]
                    )
                # v: directly [t, d] (latents stationary) - no transposes
                for tt in range(NSQ):
                    h, ub = tt // 2, tt % 2
                    base = (h % 4) * 32
                    ps_vt = psc.tile([128, 128], f32, tag="ps_vt", name="ps_vt")
                    for blk in range(8):
                        r, half = blk // 2, blk % 2
                        src = kc_r if half == 0 else vc_r
                        nc.tensor.matmul(
                            ps_vt[:],
                            lhsT=src[
                                base : base + 32, h // 4, 4 + r,
                                ub * 128 : (ub + 1) * 128,
                            ],
                            rhs=wvup_sb[base : base + 32, blk * 128 : (blk + 1) * 128],
                            start=(blk == 0),
                            stop=(blk == 7),
                            tile_position=(base, 0),
                        )
                    (nc.vector.tensor_copy if tt % 2 else nc.scalar.copy)(
                        v_sb[:, tt, :], ps_vt[:]
                    )

            # ---- phase D: transposed attention + partial o-projection ----
            # Head h+1's scores^T matmuls are interleaved into head h's
            # ones/AV loop (double pT buffer sets, by head parity) so the
            # exp's PSUM-drain latency never stalls the PE; o_proj of chunk
            # c interleaves with chunk c+1's h=0 scores the same way.
            with (
                tc.tile_pool(name="pt", bufs=1) as ptpool,
                tc.tile_pool(name="attn", bufs=2) as apool,
                tc.tile_pool(name="pss", bufs=2, space="PSUM") as pss,
                tc.tile_pool(name="pssum", bufs=1, space="PSUM") as pssum,
                tc.tile_pool(name="psav", bufs=2, space="PSUM") as psav,
                tc.tile_pool(name="pso", bufs=2, space="PSUM") as pso,
            ):
                pts = {0: [None] * NSQ, 1: [None] * NSQ}

                def emit_score(c, h, j):
                    s = h % 2
                    q0 = max(0, 128 * (j - 4 * c))
                    ps_s = pss.tile([128, CHUNK], f32, tag="ps_s", name="ps_s")
                    nc.tensor.matmul(
                        ps_s[:, q0:CHUNK],
                        lhsT=kT[:, j * 128 : (j + 1) * 128],
                        rhs=qT[:, h, c * CHUNK + q0 : (c + 1) * CHUNK],
                        start=True,
                        stop=True,
                    )
                    if j >= 4 * c:
                        # causal mask on the diagonal 128-block
                        nc.vector.tensor_add(
                            ps_s[:, q0 : q0 + 128], ps_s[:, q0 : q0 + 128], mask_sb[:]
                        )
                    pt = ptpool.tile(
                        [128, CHUNK], bf16, tag=f"pt{s}_{j}", name=f"pt{s}_{j}"
                    )
                    nc.scalar.activation(pt[:, q0:CHUNK], ps_s[:, q0:CHUNK], EXP)
                    pts[s][j] = pt

                def attn_qt(c, h, qt):
                    s = h % 2
                    i = 4 * c + qt
                    qs = slice(qt * 128, (qt + 1) * 128)
                    # softmax denominators: cheap N=1 ones-matmuls
                    ps_sum = pssum.tile([128, 1], f32, tag="ps_sum", name="ps_sum")
                    for j in range(i + 1):
                        nc.tensor.matmul(
                            ps_sum[:],
                            lhsT=pts[s][j][:, qs],
                            rhs=ones_sb[:],
                            start=(j == 0),
                            stop=(j == i),
                        )
                    rec = apool.tile([128, 1], f32, tag="rec", name="rec")
                    nc.vector.reciprocal(rec[:], ps_sum[:])
                    # AV in [q, d]: normalization becomes a per-partition
                    # scale on the PSUM->SBUF copy
                    ps_av = psav.tile([128, 128], f32, tag="ps_av", name="ps_av")
                    for j in range(i + 1):
                        nc.tensor.matmul(
                            ps_av[:],
                            lhsT=pts[s][j][:, qs],
                            rhs=v_sb[:, j, :],
                            start=(j == 0),
                            stop=(j == i),
                        )
                    at_qd = apool.tile(
                        [128, 128], bf16, tag="at_qd", name="at_qd", bufs=3
                    )
                    nc.vector.tensor_scalar_mul(at_qd[:], ps_av[:], rec[:])
                    return at_qd

                def emit_transpose(at_sb_t, h, qt, at_qd):
                    qs = slice(qt * 128, (qt + 1) * 128)
                    ps_tr = psav.tile(
                        [128, 128], bf16, tag="ps_tr", name="ps_tr", bufs=1
                    )
                    nc.tensor.transpose(ps_tr[:], at_qd[:], id_sb[:])
                    (nc.scalar.copy if qt % 2 else nc.vector.tensor_copy)(
                        at_sb_t[:, h, qs], ps_tr[:]
                    )

                for c in range(NCHUNK):
                    nj = 4 * c + 4
                    at_sb = apool.tile(
                        [128, HPC, CHUNK], bf16, tag="at_sb", name="at_sb"
                    )
                    if c == 0:
                        for j in range(nj):
                            emit_score(0, 0, j)
                    for h in range(HPC):
                        njs = list(range(nj)) if h + 1 < HPC else []
                        pending = None
                        for qt in range(4):
                            at_qd = attn_qt(c, h, qt)
                            lo = (qt * len(njs)) // 4
                            hi = ((qt + 1) * len(njs)) // 4
                            for j in njs[lo:hi]:
                                emit_score(c, h + 1, j)
                            if pending is not None:
                                emit_transpose(at_sb, h, pending[0], pending[1])
                            pending = (qt, at_qd)
                        emit_transpose(at_sb, h, pending[0], pending[1])
                    # o-projection for this chunk, interleaving next chunk's
                    # h=0 scores
                    next_js = list(range(4 * (c + 1) + 4)) if c + 1 < NCHUNK else []
                    for qt in range(4):
                        i = 4 * c + qt
                        for pn in range(HID // (2 * CHUNK)):
                            out_sb = opool.tile(
                                [128, 2 * CHUNK], f32, tag="out_sb", name="out_sb"
                            )
                            for half in range(2):
                                n = 2 * pn + half
                                ps_o = pso.tile(
                                    [128, CHUNK], f32, tag="ps_o", name="ps_o"
                                )
                                for hh in range(HPC):
                                    nc.tensor.matmul(
                                        ps_o[:],
                                        lhsT=at_sb[:, hh, qt * 128 : (qt + 1) * 128],
                                        rhs=wo_sb[:, hh, n * CHUNK : (n + 1) * CHUNK],
                                        start=(hh == 0),
                                        stop=(hh == HPC - 1),
                                    )
                                (nc.scalar.copy if half else nc.vector.tensor_copy)(
                                    out_sb[:, half * CHUNK : (half + 1) * CHUNK],
                                    ps_o[:],
                                )
                            (nc.sync if pn % 2 == 0 else nc.gpsimd).dma_start(
                                out=outp[
                                    i * 128 : (i + 1) * 128,
                                    2 * pn * CHUNK : 2 * (pn + 1) * CHUNK,
                                ],
                                in_=out_sb[:],
                            )
                        lo = (qt * len(next_js)) // 4
                        hi = ((qt + 1) * len(next_js)) // 4
                        for j in next_js[lo:hi]:
                            emit_score(c + 1, 0, j)
    nc.compile()
    return nc


def _prep_inputs(hidden_states, Wq, Wk_down, Wv_down, Wk_up, Wv_up, Wo):
    bf = ml_dtypes.bfloat16
    hs = np.asarray(hidden_states, dtype=np.float32).reshape(S, HID)
    ht = np.ascontiguousarray(hs.T).astype(bf)
    scale = np.float32(1.0) / np.sqrt(np.float32(D))
    Wq = np.asarray(Wq, dtype=np.float32)
    Wo = np.asarray(Wo, dtype=np.float32)
    wkdt = np.ascontiguousarray(np.asarray(Wk_down, np.float32).T).astype(bf)
    wvdt = np.ascontiguousarray(np.asarray(Wv_down, np.float32).T).astype(bf)
    # transposed causal mask: rows t, cols q; allowed where q >= t
    mask = np.where(
        np.arange(128)[None, :] >= np.arange(128)[:, None], 0.0, NEG
    ).astype(np.float32)
    identity = np.eye(128, dtype=bf)

    def up_blocks(w):  # w: (128, 256) rows of Wk_up/Wv_up for this core
        arr = np.zeros((128, 8 * 128), np.float32)
        for r in range(4):
            for half in range(2):
                blk = r * 2 + half
                bT = w[:, r * 64 + half * 32 : r * 64 + half * 32 + 32].T
                for b in range(4):
                    arr[b * 32 : (b + 1) * 32, blk * 128 : (blk + 1) * 128] = bT
        return arr.astype(bf)

    in_maps = []
    for c in range(NCORE):
        htm = np.ascontiguousarray(ht[:, c * SSH : (c + 1) * SSH])
        wqt = np.ascontiguousarray((Wq[c * 512 : (c + 1) * 512, :] * scale).T).astype(
            bf
        )
        wkup = up_blocks(np.asarray(Wk_up[c * 128 : (c + 1) * 128, :], np.float32))
        wvup = up_blocks(np.asarray(Wv_up[c * 128 : (c + 1) * 128, :], np.float32))
        wot = np.ascontiguousarray(Wo[:, c * 512 : (c + 1) * 512].T).astype(bf)
        in_maps.append(
            dict(
                ht=ht,
                htm=htm,
                wqt=wqt,
                wkdt=wkdt,
                wvdt=wvdt,
                wkup=wkup,
                wvup=wvup,
                wot=wot,
                maskt=mask,
                ident=identity,
            )
        )
    return in_maps


def run(trace=False, **inputs):
    from concourse.bass_utils import run_bass_kernel_spmd

    global _BUILT
    if _BUILT is None:
        _BUILT = _build()
    in_maps = _prep_inputs(**inputs)
    res = run_bass_kernel_spmd(
        _BUILT, in_maps, core_ids=list(range(NCORE)), trace=trace
    )
    acc = np.array(res.results[0]["out"], dtype=np.float32, copy=True)
    for r in res.results[1:]:
        acc += np.asarray(r["out"], dtype=np.float32)
    return acc.reshape(B, S, HID), res


def kernel(**inputs):
    out, _ = run(trace=False, **inputs)
    return out


# revision 8
# speedup vs baseline: 1.0828x; 1.0160x over previous
"""Multi-Head Latent Attention (GQA, causal) on 8 Trainium2 NeuronCores.

Sharding: tensor-parallel by heads. Core c owns query heads 4c..4c+3 and
kv head c. Each core computes:
  - its slice of the q projection (output dims c*512..(c+1)*512),
  - its S/8 sequence shard of the kc/vc down-projections, AllGathered so
    every core sees the full compressed latents (the reference's scrambled
    latent reshape is folded into strided access patterns),
  - its kv head's up-projections (k in [d,t] layout, v directly in [t,d]),
  - head-parallel causal attention computed TRANSPOSED: scores^T[t,q] come
    straight out of the PE in the layout the AV matmul consumes, so no
    per-block P transposes are needed. Softmax denominators are N=1
    ones-matmuls; normalization is folded into the AV output copy, and one
    128x128 PE transpose per (head, q-tile) restores [d,q] for o_proj,
  - a partial o-projection (input dims c*512..(c+1)*512) over the full
    hidden size.
The host sums the 8 partial outputs (the all-reduce after o_proj).
"""

import sys

import ml_dtypes
import numpy as np

if "/opt/trn_rl_repo" not in sys.path:
    sys.path.insert(0, "/opt/trn_rl_repo")

B, S, HID = 1, 2048, 4096
H, HK, D = 32, 8, 128
L = D // 4  # 32
NCORE = 8
HPC = H // NCORE  # 4 query heads per core
NKT = HID // 128  # 32 k-tiles over hidden dim
CHUNK = 512
NCHUNK = S // CHUNK  # 4
NSQ = S // 128  # 16 sq tiles
SSH = S // NCORE  # 256
NEG = -1e9

_BUILT = None


def _build():
    import concourse.mybir as mybir
    import concourse.tile as tile
    from concourse import bacc

    f32 = mybir.dt.float32
    bf16 = mybir.dt.bfloat16
    EXP = mybir.ActivationFunctionType.Exp

    nc = bacc.Bacc()

    ht = nc.dram_tensor("ht", [HID, S], bf16, kind="ExternalInput")
    htm = nc.dram_tensor("htm", [HID, SSH], bf16, kind="ExternalInput")
    wqt = nc.dram_tensor("wqt", [HID, HPC * D], bf16, kind="ExternalInput")
    wkdt = nc.dram_tensor("wkdt", [HID, HK * L], bf16, kind="ExternalInput")
    wvdt = nc.dram_tensor("wvdt", [HID, HK * L], bf16, kind="ExternalInput")
    wkup = nc.dram_tensor("wkup", [128, 8 * D], bf16, kind="ExternalInput")
    wvup = nc.dram_tensor("wvup", [128, 8 * D], bf16, kind="ExternalInput")
    wot = nc.dram_tensor("wot", [HPC * D, HID], bf16, kind="ExternalInput")
    maskt = nc.dram_tensor("maskt", [128, 128], f32, kind="ExternalInput")
    ident = nc.dram_tensor("ident", [128, 128], bf16, kind="ExternalInput")
    outp = nc.dram_tensor("out", [S, HID], f32, kind="ExternalOutput")
    # kc/vc shard exchange: [p, tgt*512 + m*256 + u] per core -> gathered
    cv_bounce = nc.dram_tensor("cv_bounce", [128, 1024], bf16)
    cv_gath = nc.dram_tensor("cv_gath", [NCORE, 128, 1024], bf16, addr_space="Shared")

    with tile.TileContext(nc) as tc:
        with (
            tc.tile_pool(name="weights", bufs=1) as wpool,
            tc.tile_pool(name="persist", bufs=1) as ppool,
            tc.tile_pool(name="stream", bufs=6) as spool,
            tc.tile_pool(name="outs", bufs=3) as opool,
        ):
            # ---- constants + resident weights ----
            ones_sb = wpool.tile([128, 1], bf16)
            nc.gpsimd.memset(ones_sb[:], 1.0)
            mask_sb = wpool.tile([128, 128], f32)
            nc.scalar.dma_start(out=mask_sb[:], in_=maskt[:])
            id_sb = wpool.tile([128, 128], bf16)
            nc.scalar.dma_start(out=id_sb[:], in_=ident[:])
            wkup_sb = wpool.tile([128, 8 * D], bf16)
            nc.scalar.dma_start(out=wkup_sb[:], in_=wkup[:])
            wvup_sb = wpool.tile([128, 8 * D], bf16)
            nc.scalar.dma_start(out=wvup_sb[:], in_=wvup[:])
            wq_sb = wpool.tile([128, NKT, HPC * D], bf16)
            wq_r = wqt.rearrange("(k p) c -> p k c", p=128)
            for g in range(4):
                ks = slice(g * 4, (g + 1) * 4)
                nc.scalar.dma_start(out=wq_sb[:, ks, :], in_=wq_r[:, ks, :])
            wo_sb = wpool.tile([128, HPC, HID], bf16)
            wo_r = wot.rearrange("(k p) c -> p k c", p=128)

            # ---- persistent activations ----
            qT = ppool.tile([128, HPC, S], bf16)  # [d, head, s]
            kcT = ppool.tile([128, 2, S], bf16)  # [latent%128, latent//128, s]
            vcT = ppool.tile([128, 2, S], bf16)
            kT = ppool.tile([128, S], bf16)  # [d, t] for our kv head
            # [t%128, t//128, d]; col 128 is a constant 1.0 column so the
            # AV matmul also produces the softmax denominator (col 128 of out)
            v_sb = ppool.tile([128, NSQ, 132], bf16)
            nc.gpsimd.memset(v_sb[:, :, 128:129], 1.0)

            ht_r = ht.rearrange("(k p) s -> p k s", p=128)

            # ---- phase B0 + q chunk 0, interleaved at k-tile grain:
            #      the cv shard matmuls and q chunk 0 share the DMA window so
            #      the PE is never starved while weights stream in ----
            with tc.tile_pool(name="psq", bufs=1, space="PSUM") as psq:

                def q_chunk_mms(sc, ps_q, kp, hook=None):
                    hch = spool.tile([128, 2, CHUNK], bf16, tag="hch", name="hch")
                    nc.gpsimd.dma_start(
                        out=hch[:],
                        in_=ht_r[
                            :, 2 * kp : 2 * kp + 2, sc * CHUNK : (sc + 1) * CHUNK
                        ],
                    )
                    for kk in range(2):
                        k = 2 * kp + kk
                        if hook is not None:
                            hook(k)
                        st = dict(start=(k == 0), stop=(k == NKT - 1))
                        for m in range(HPC):
                            nc.tensor.matmul(
                                ps_q[m][:],
                                lhsT=wq_sb[:, k, m * 128 : (m + 1) * 128],
                                rhs=hch[:, kk, :],
                                **st,
                            )

                def q_copies(sc, ps_q):
                    cs = slice(sc * CHUNK, (sc + 1) * CHUNK)
                    for m in range(HPC):
                        (nc.scalar.copy if m % 2 else nc.vector.tensor_copy)(
                            qT[:, m, cs], ps_q[m][:]
                        )

                with (
                    tc.tile_pool(name="b0", bufs=1) as bpool,
                    tc.tile_pool(name="psb0", bufs=1, space="PSUM") as psb0,
                ):
                    wkd_sb = bpool.tile([128, NKT, HK * L], bf16)
                    wvd_sb = bpool.tile([128, NKT, HK * L], bf16)
                    wkd_r = wkdt.rearrange("(k p) c -> p k c", p=128)
                    wvd_r = wvdt.rearrange("(k p) c -> p k c", p=128)
                    for g in range(8):
                        ks = slice(g * 4, (g + 1) * 4)
                        nc.sync.dma_start(out=wkd_sb[:, ks, :], in_=wkd_r[:, ks, :])
                        nc.sync.dma_start(out=wvd_sb[:, ks, :], in_=wvd_r[:, ks, :])
                    hm = bpool.tile([128, NKT, SSH], bf16)
                    hm_r = htm.rearrange("(k p) s -> p k s", p=128)
                    for g in range(4):
                        ks = slice(g * 8, (g + 1) * 8)
                        nc.gpsimd.dma_start(out=hm[:, ks, :], in_=hm_r[:, ks, :])
                    ps_cv = [
                        psb0.tile([128, SSH], f32, tag=f"ps_cv{t}", name=f"ps_cv{t}")
                        for t in range(4)
                    ]
                    ps_q0 = [
                        psq.tile([128, CHUNK], f32, tag=f"ps_q{m}", name=f"ps_q{m}")
                        for m in range(HPC)
                    ]

                    def cv_mms(k):
                        for ti, wsb_d in ((0, wkd_sb), (1, wvd_sb)):
                            for m in range(2):
                                nc.tensor.matmul(
                                    ps_cv[ti * 2 + m][:],
                                    lhsT=wsb_d[:, k, m * 128 : (m + 1) * 128],
                                    rhs=hm[:, k, :],
                                    start=(k == 0),
                                    stop=(k == NKT - 1),
                                )

                    for kp in range(NKT // 2):
                        q_chunk_mms(0, ps_q0, kp, hook=cv_mms)
                    cvst = bpool.tile([128, 1024], bf16)
                    for t in range(4):
                        ti, m = t // 2, t % 2
                        eng = nc.vector.tensor_copy if t % 2 == 0 else nc.scalar.copy
                        eng(
                            cvst[:, ti * 512 + m * 256 : ti * 512 + (m + 1) * 256],
                            ps_cv[t][:],
                        )
                    nc.sync.dma_start(out=cv_bounce[:], in_=cvst[:])
                    q_copies(0, ps_q0)
                for g in range(4, 8):
                    ks = slice(g * 4, (g + 1) * 4)
                    nc.scalar.dma_start(out=wq_sb[:, ks, :], in_=wq_r[:, ks, :])

                nc.gpsimd.collective_compute(
                    "AllGather",
                    mybir.AluOpType.bypass,
                    replica_groups=[list(range(NCORE))],
                    ins=[cv_bounce[:]],
                    outs=[cv_gath[:]],
                )
                g_r = cv_gath.rearrange("r p (t m u) -> t p m r u", t=2, m=2)
                for m in range(2):
                    nc.sync.dma_start(
                        out=kcT[:, m, :].rearrange("p (r u) -> p r u", r=NCORE),
                        in_=g_r[0, :, m],
                    )
                    nc.sync.dma_start(
                        out=vcT[:, m, :].rearrange("p (r u) -> p r u", r=NCORE),
                        in_=g_r[1, :, m],
                    )
                # wo behind the gathers on the sync queue: issues once the
                # collective lands, transfers while the DMA engines are quiet
                for g in range(HPC):
                    nc.sync.dma_start(out=wo_sb[:, g, :], in_=wo_r[:, g, :])
                for sc in range(1, NCHUNK):
                    ps_q = [
                        psq.tile([128, CHUNK], f32, tag=f"ps_q{m}", name=f"ps_q{m}")
                        for m in range(HPC)
                    ]
                    for kp in range(NKT // 2):
                        q_chunk_mms(sc, ps_q, kp)
                    q_copies(sc, ps_q)

            # ---- phase C: up projections ----
            # k_cmp[t, c'] with t = h*256+u, c' = r*64 + half*32 + j maps to
            #   (half==0 ? KC : VC)[8u + r (+4 for v_cmp), h*32 + j]
            # so the latent operand is a stride-8 slice of kcT/vcT along seq.
            kc_r = kcT.rearrange("p m (u r) -> p m r u", r=8)
            vc_r = vcT.rearrange("p m (u r) -> p m r u", r=8)
            with tc.tile_pool(name="psc", bufs=2, space="PSUM") as psc:
                # k: [d, t] (weights stationary)
                for h in range(8):
                    base = (h % 4) * 32
                    ps_up = psc.tile([128, 256], f32, tag="ps_up", name="ps_up")
                    for blk in range(8):
                        r, half = blk // 2, blk % 2
                        src = kc_r if half == 0 else vc_r
                        nc.tensor.matmul(
                            ps_up[:],
                            lhsT=wkup_sb[base : base + 32, blk * 128 : (blk + 1) * 128],
                            rhs=src[base : base + 32, h // 4, r, :],
                            start=(blk == 0),
                            stop=(blk == 7),
                            tile_position=(base, 0),
                        )
                    (nc.vector.tensor_copy if h % 2 else nc.scalar.copy)(
                        kT[:, h * 256 : (h + 1) * 256], ps_up[:]
                    )
                # v: directly [t, d] (latents stationary) - no transposes
                for tt in range(NSQ):
                    h, ub = tt // 2, tt % 2
                    base = (h % 4) * 32
                    ps_vt = psc.tile([128, 128], f32, tag="ps_vt", name="ps_vt")
                    for blk in range(8):
                        r, half = blk // 2, blk % 2
                        src = kc_r if half == 0 else vc_r
                        nc.tensor.matmul(
                            ps_vt[:],
                            lhsT=src[
                                base : base + 32, h // 4, 4 + r,
                                ub * 128 : (ub + 1) * 128,
                            ],
                            rhs=wvup_sb[base : base + 32, blk * 128 : (blk + 1) * 128],
                            start=(blk == 0),
                            stop=(blk == 7),
                            tile_position=(base, 0),
                        )
                    (nc.vector.tensor_copy if tt % 2 else nc.scalar.copy)(
                        v_sb[:, tt, 0:128], ps_vt[:]
                    )

            # ---- phase D: transposed attention + partial o-projection ----
            # Head h+1's scores^T matmuls are interleaved into head h's
            # ones/AV loop (double pT buffer sets, by head parity) so the
            # exp's PSUM-drain latency never stalls the PE; o_proj of chunk
            # c interleaves with chunk c+1's h=0 scores the same way.
            with (
                tc.tile_pool(name="pt", bufs=1) as ptpool,
                tc.tile_pool(name="attn", bufs=2) as apool,
                tc.tile_pool(name="pss", bufs=3, space="PSUM") as pss,
                tc.tile_pool(name="psav", bufs=2, space="PSUM") as psav,
                tc.tile_pool(name="pso", bufs=2, space="PSUM") as pso,
            ):
                pts = {0: [None] * NSQ, 1: [None] * NSQ}

                def emit_score(c, h, j):
                    s = h % 2
                    q0 = max(0, 128 * (j - 4 * c))
                    ps_s = pss.tile([128, CHUNK], f32, tag="ps_s", name="ps_s")
                    nc.tensor.matmul(
                        ps_s[:, q0:CHUNK],
                        lhsT=kT[:, j * 128 : (j + 1) * 128],
                        rhs=qT[:, h, c * CHUNK + q0 : (c + 1) * CHUNK],
                        start=True,
                        stop=True,
                    )
                    if j >= 4 * c:
                        # causal mask on the diagonal 128-block
                        nc.vector.tensor_add(
                            ps_s[:, q0 : q0 + 128], ps_s[:, q0 : q0 + 128], mask_sb[:]
                        )
                    pt = ptpool.tile(
                        [128, CHUNK], bf16, tag=f"pt{s}_{j}", name=f"pt{s}_{j}"
                    )
                    nc.scalar.activation(pt[:, q0:CHUNK], ps_s[:, q0:CHUNK], EXP)
                    pts[s][j] = pt

                def attn_qt(c, h, qt):
                    s = h % 2
                    i = 4 * c + qt
                    qs = slice(qt * 128, (qt + 1) * 128)
                    # AV in [q, d|sum]: col 128 accumulates the softmax
                    # denominator via v_sb's ones column; normalization is a
                    # per-partition scale on the PSUM->SBUF copy
                    ps_av = psav.tile([128, 129], f32, tag="ps_av", name="ps_av")
                    for j in range(i + 1):
                        nc.tensor.matmul(
                            ps_av[:],
                            lhsT=pts[s][j][:, qs],
                            rhs=v_sb[:, j, 0:129],
                            start=(j == 0),
                            stop=(j == i),
                        )
                    rec = apool.tile([128, 1], f32, tag="rec", name="rec")
                    nc.vector.reciprocal(rec[:], ps_av[:, 128:129])
                    at_qd = apool.tile(
                        [128, 128], bf16, tag="at_qd", name="at_qd", bufs=3
                    )
                    nc.vector.tensor_scalar_mul(at_qd[:], ps_av[:, 0:128], rec[:])
                    return at_qd

                def emit_transpose(at_sb_t, h, qt, at_qd):
                    qs = slice(qt * 128, (qt + 1) * 128)
                    ps_tr = psav.tile(
                        [128, 128], bf16, tag="ps_tr", name="ps_tr", bufs=1
                    )
                    nc.tensor.transpose(ps_tr[:], at_qd[:], id_sb[:])
                    (nc.scalar.copy if qt % 2 else nc.vector.tensor_copy)(
                        at_sb_t[:, h, qs], ps_tr[:]
                    )

                for c in range(NCHUNK):
                    nj = 4 * c + 4
                    at_sb = apool.tile(
                        [128, HPC, CHUNK], bf16, tag="at_sb", name="at_sb"
                    )
                    if c == 0:
                        for j in range(nj):
                            emit_score(0, 0, j)
                    for h in range(HPC):
                        njs = list(range(nj)) if h + 1 < HPC else []
                        pending = None
                        for qt in range(4):
                            at_qd = attn_qt(c, h, qt)
                            lo = (qt * len(njs)) // 4
                            hi = ((qt + 1) * len(njs)) // 4
                            for j in njs[lo:hi]:
                                emit_score(c, h + 1, j)
                            if pending is not None:
                                emit_transpose(at_sb, h, pending[0], pending[1])
                            pending = (qt, at_qd)
                        emit_transpose(at_sb, h, pending[0], pending[1])
                    # o-projection for this chunk, interleaving next chunk's
                    # h=0 scores
                    next_js = list(range(4 * (c + 1) + 4)) if c + 1 < NCHUNK else []
                    for qt in range(4):
                        i = 4 * c + qt
                        for pn in range(HID // (2 * CHUNK)):
                            out_sb = opool.tile(
                                [128, 2 * CHUNK], f32, tag="out_sb", name="out_sb"
                            )
                            for half in range(2):
                                n = 2 * pn + half
                                ps_o = pso.tile(
                                    [128, CHUNK], f32, tag="ps_o", name="ps_o"
                                )
                                for hh in range(HPC):
                                    nc.tensor.matmul(
                                        ps_o[:],
                                        lhsT=at_sb[:, hh, qt * 128 : (qt + 1) * 128],
                                        rhs=wo_sb[:, hh, n * CHUNK : (n + 1) * CHUNK],
                                        start=(hh == 0),
                                        stop=(hh == HPC - 1),
                                    )
                                (nc.scalar.copy if half else nc.vector.tensor_copy)(
                                    out_sb[:, half * CHUNK : (half + 1) * CHUNK],
                                    ps_o[:],
                                )
                            (nc.sync if pn % 2 == 0 else nc.gpsimd).dma_start(
                                out=outp[
                                    i * 128 : (i + 1) * 128,
                                    2 * pn * CHUNK : 2 * (pn + 1) * CHUNK,
                                ],
                                in_=out_sb[:],
                            )
                        lo = (qt * len(next_js)) // 4
                        hi = ((qt + 1) * len(next_js)) // 4
                        for j in next_js[lo:hi]:
                            emit_score(c + 1, 0, j)
    nc.compile()
    return nc


def _prep_inputs(hidden_states, Wq, Wk_down, Wv_down, Wk_up, Wv_up, Wo):
    bf = ml_dtypes.bfloat16
    hs = np.asarray(hidden_states, dtype=np.float32).reshape(S, HID)
    ht = np.ascontiguousarray(hs.T).astype(bf)
    scale = np.float32(1.0) / np.sqrt(np.float32(D))
    Wq = np.asarray(Wq, dtype=np.float32)
    Wo = np.asarray(Wo, dtype=np.float32)
    wkdt = np.ascontiguousarray(np.asarray(Wk_down, np.float32).T).astype(bf)
    wvdt = np.ascontiguousarray(np.asarray(Wv_down, np.float32).T).astype(bf)
    # transposed causal mask: rows t, cols q; allowed where q >= t
    mask = np.where(
        np.arange(128)[None, :] >= np.arange(128)[:, None], 0.0, NEG
    ).astype(np.float32)
    identity = np.eye(128, dtype=bf)

    def up_blocks(w):  # w: (128, 256) rows of Wk_up/Wv_up for this core
        arr = np.zeros((128, 8 * 128), np.float32)
        for r in range(4):
            for half in range(2):
                blk = r * 2 + half
                bT = w[:, r * 64 + half * 32 : r * 64 + half * 32 + 32].T
                for b in range(4):
                    arr[b * 32 : (b + 1) * 32, blk * 128 : (blk + 1) * 128] = bT
        return arr.astype(bf)

    in_maps = []
    for c in range(NCORE):
        htm = np.ascontiguousarray(ht[:, c * SSH : (c + 1) * SSH])
        wqt = np.ascontiguousarray((Wq[c * 512 : (c + 1) * 512, :] * scale).T).astype(
            bf
        )
        wkup = up_blocks(np.asarray(Wk_up[c * 128 : (c + 1) * 128, :], np.float32))
        wvup = up_blocks(np.asarray(Wv_up[c * 128 : (c + 1) * 128, :], np.float32))
        wot = np.ascontiguousarray(Wo[:, c * 512 : (c + 1) * 512].T).astype(bf)
        in_maps.append(
            dict(
                ht=ht,
                htm=htm,
                wqt=wqt,
                wkdt=wkdt,
                wvdt=wvdt,
                wkup=wkup,
                wvup=wvup,
                wot=wot,
                maskt=mask,
                ident=identity,
            )
        )
    return in_maps


def run(trace=False, **inputs):
    from concourse.bass_utils import run_bass_kernel_spmd

    global _BUILT
    if _BUILT is None:
        _BUILT = _build()
    in_maps = _prep_inputs(**inputs)
    res = run_bass_kernel_spmd(
        _BUILT, in_maps, core_ids=list(range(NCORE)), trace=trace
    )
    acc = np.array(res.results[0]["out"], dtype=np.float32, copy=True)
    for r in res.results[1:]:
        acc += np.asarray(r["out"], dtype=np.float32)
    return acc.reshape(B, S, HID), res


def kernel(**inputs):
    out, _ = run(trace=False, **inputs)
    return out
